# revision 15
# baseline (speedup 1.0000x reference)
"""CenterLoss Trainium2 kernel (raw bacc, explicit semaphores).

loss = mean_i clip(||features_i - centers[target_i]||^2, 1e-12, 1e12)
       + (NUM_CLASSES-1) * 1e-12        # the clipped zeros of the masked distmat

The reference builds the full [8192, 2048] distance matrix and masks out
everything but the target column; only the per-row target distance matters,
so the kernel is a gather + (f-c)^2-reduce:

  - data-parallel over the batch: 1024 rows per core on 8 cores
  - centers stay in HBM; per slot g (128 rows, one per partition) an
    indirect SWDGE DMA gathers centers[idx[p, g]] -> c_t[p, g*512:...]
  - DVE computes diff = f - c per slot; ACT squares with fused
    per-partition accumulate into acc[:, g]
  - the per-core [128, 8] partial tiles are summed on the host (the
    "all-reduce" of the scalar loss)

Layout per core: shard row r (0..1023) lives at partition r // 8, slot
r % 8 (the natural contiguous [1024, 512] -> [128, 8*512] reshape);
idx[p, g] = target[8p + g].

Ordering notes (from profiling):
  - the idx load goes first and the feature loads wait for its semaphore —
    otherwise the tiny idx transfer's 16 sem increments trickle out behind
    2 MB of feature packets in the SDMA round-robin and gate the gathers
    ~10 us late
  - indirect_dma_start (InstDMACopy + dynamic AP) gathers one row per
    partition per call; per-call cost is ~1.1 us of Q7 descgen, no
    extended-instruction library load (dma_gather would stall ~6 us on
    LOAD_LIB ucode fetch)
  - ACT's Square is bit-exact for f32 (measured: elementwise == f32
    multiply, accum == f32 sequential sum)
"""

from contextlib import ExitStack

import numpy as np

import concourse.bacc as bacc
import concourse.bass as bass
from concourse import mybir
from concourse.bass_utils import run_bass_kernel_spmd

N_CORES = 8
BATCH = 8192
FEAT = 512
NCLS = 2048
P = 128

ROWS = BATCH // N_CORES          # 1024 rows per core
SLOTS = ROWS // P                # 8 rows per partition = 8 gather calls
FREE = SLOTS * FEAT              # 4096 f32 per partition
FHALF = FREE // 2                # feature DMA granularity (2 x 1 MB)

_CACHE: dict[str, object] = {}

F32 = mybir.dt.float32


def _build_nc():
    nc = bacc.Bacc(
        "TRN2", target_bir_lowering=False, debug=False, enable_asserts=False
    )

    feats = nc.dram_tensor("features", [P, FREE], F32, kind="ExternalInput")
    centers = nc.dram_tensor("centers", [NCLS, FEAT], F32, kind="ExternalInput")
    idxs = nc.dram_tensor("idxs", [P, SLOTS], mybir.dt.int32, kind="ExternalInput")
    partials = nc.dram_tensor("partials", [P, SLOTS], F32, kind="ExternalOutput")

    with (
        nc.sbuf_tensor("f_t", [P, FREE], F32) as f_t,
        nc.sbuf_tensor("c_t", [P, FREE], F32) as c_t,
        nc.sbuf_tensor("d_t", [P, FREE], F32) as d_t,
        nc.sbuf_tensor("idx_t", [P, SLOTS], mybir.dt.int32) as idx_t,
        nc.sbuf_tensor("acc", [P, SLOTS], F32) as acc,
        nc.semaphore("s_idx") as s_idx,
        nc.semaphore("s_f0") as s_f0,
        nc.semaphore("s_f1") as s_f1,
        nc.semaphore("s_sub") as s_sub,
        nc.semaphore("s_sq") as s_sq,
        nc.semaphore("s_out") as s_out,
        ExitStack() as stack,
    ):
        # one semaphore per gather DMA: a shared counting sem is racy —
        # per-SDMA-engine completion skew means a cumulative count can hit
        # 16*(g+1) while some engine still owes call g's last bytes
        s_gath = [
            stack.enter_context(nc.semaphore(f"s_g{g}")) for g in range(SLOTS)  # noqa: ANT232
        ]
        s_feat = [s_f0, s_f1]
        block = stack.enter_context(nc.Block())

        @block.sync
        def _(sync: bass.BassEngine):
            for h in range(2):
                sync.dma_start(
                    f_t[:, h * FHALF:(h + 1) * FHALF],
                    feats[:, h * FHALF:(h + 1) * FHALF],
                ).then_inc(s_feat[h], 16)
            sync.wait_ge(s_sq, SLOTS)
            sync.dma_start(partials[:], acc[:]).then_inc(s_out, 16)
            sync.wait_ge(s_out, 16)

        @block.gpsimd
        def _(gpsimd: bass.BassGpSimd):
            # idx via gpsimd's own SWDGE queue: issues ~1 us earlier than the
            # HWDGE path and keeps the feature ring free of idx packets; the
            # gathers that need it run on this same engine right after
            gpsimd.dma_start(idx_t[:], idxs[:]).then_inc(s_idx, 16)
            gpsimd.wait_ge(s_idx, 16)
            for g in range(SLOTS):
                gpsimd.indirect_dma_start(
                    out=c_t[:, g * FEAT:(g + 1) * FEAT],
                    out_offset=None,
                    in_=centers[:],
                    in_offset=bass.IndirectOffsetOnAxis(
                        ap=idx_t[:, g:g + 1], axis=0
                    ),
                ).then_inc(s_gath[g], 16)

        @block.vector
        def _(vector: bass.BassEngine):
            for g in range(SLOTS):
                vector.wait_ge(s_gath[g], 16)
                vector.wait_ge(s_feat[g // (SLOTS // 2)], 16)
                vector.tensor_tensor(
                    out=d_t[:, g * FEAT:(g + 1) * FEAT],
                    in0=f_t[:, g * FEAT:(g + 1) * FEAT],
                    in1=c_t[:, g * FEAT:(g + 1) * FEAT],
                    op=mybir.AluOpType.subtract,
                ).then_inc(s_sub, 1)

        @block.scalar
        def _(scalar: bass.BassEngine):
            for g in range(SLOTS):
                scalar.wait_ge(s_sub, g + 1)
                # in-place square: ACT streams read-before-write per element
                scalar.activation(
                    out=d_t[:, g * FEAT:(g + 1) * FEAT],
                    in_=d_t[:, g * FEAT:(g + 1) * FEAT],
                    func=mybir.ActivationFunctionType.Square,
                    accum_out=acc[:, g:g + 1],
                ).then_inc(s_sq, 1)

    nc.compile()
    return nc


def _get_nc():
    if "nc" not in _CACHE:
        _CACHE["nc"] = _build_nc()
    return _CACHE["nc"]


def _prep_inputs(features: np.ndarray, centers: np.ndarray, target: np.ndarray):
    """Shard host-side. Core i takes rows [1024*i, 1024*(i+1)). Within a
    core, rows are ordered by target class and rank k goes to partition
    k % 128, slot k // 128 — each gather call then reads 128 consecutive
    sorted indices, a narrow mostly-sequential window of the centers table
    (much friendlier HBM access than random 2 KB reads)."""
    feats_f32 = np.ascontiguousarray(features, dtype=np.float32).reshape(
        N_CORES, ROWS, FEAT
    )
    tgt = target.astype(np.int32).reshape(N_CORES, ROWS)
    cent = np.ascontiguousarray(centers, dtype=np.float32)

    feats = np.empty((N_CORES, P, FREE), dtype=np.float32)
    idx = np.empty((N_CORES, P, SLOTS), dtype=np.int32)
    for i in range(N_CORES):
        order = np.argsort(tgt[i], kind="stable")
        # rank k -> partition k % P, slot k // P
        feats[i] = (
            feats_f32[i][order].reshape(SLOTS, P, FEAT).transpose(1, 0, 2).reshape(P, FREE)
        )
        idx[i] = tgt[i][order].reshape(SLOTS, P).T
    return feats, cent, idx


def kernel(features: np.ndarray, centers: np.ndarray, target: np.ndarray) -> np.ndarray:
    nc = _get_nc()
    feats, cent, idx = _prep_inputs(features, centers, target)

    in_maps = [
        {"features": feats[i], "centers": cent, "idxs": idx[i]}
        for i in range(N_CORES)
    ]
    res = run_bass_kernel_spmd(nc, in_maps, core_ids=list(range(N_CORES)))

    total = 0.0
    for r in res.results:
        total += float(r["partials"].astype(np.float64).sum())
    loss = total / BATCH + (NCLS - 1) * 1e-12
    return np.asarray(loss, dtype=np.float32)


# revision 16
# speedup vs baseline: 1.1075x; 1.1075x over previous
"""CenterLoss Trainium2 kernel (raw bacc, explicit semaphores).

loss = mean_i clip(||features_i - centers[target_i]||^2, 1e-12, 1e12)
       + (NUM_CLASSES-1) * 1e-12        # the clipped zeros of the masked distmat

The reference builds the full [8192, 2048] distance matrix and masks out
everything but the target column; only the per-row target distance matters,
so the kernel is a gather + (f-c)^2-reduce:

  - data-parallel over the batch: 1024 rows per core on 8 cores
  - centers stay in HBM; per slot g (128 rows, one per partition) an
    indirect SWDGE DMA gathers centers[idx[p, g]] -> c_t[p, g*512:...]
  - DVE computes diff = f - c per slot; ACT squares with fused
    per-partition accumulate into acc[:, g]
  - the per-core [128, 8] partial tiles are summed on the host (the
    "all-reduce" of the scalar loss)

Layout per core: shard row r (0..1023) lives at partition r // 8, slot
r % 8 (the natural contiguous [1024, 512] -> [128, 8*512] reshape);
idx[p, g] = target[8p + g].

Ordering notes (from profiling):
  - the idx load goes first and the feature loads wait for its semaphore —
    otherwise the tiny idx transfer's 16 sem increments trickle out behind
    2 MB of feature packets in the SDMA round-robin and gate the gathers
    ~10 us late
  - indirect_dma_start (InstDMACopy + dynamic AP) gathers one row per
    partition per call; per-call cost is ~1.1 us of Q7 descgen, no
    extended-instruction library load (dma_gather would stall ~6 us on
    LOAD_LIB ucode fetch)
  - ACT's Square is bit-exact for f32 (measured: elementwise == f32
    multiply, accum == f32 sequential sum)
"""

from contextlib import ExitStack

import numpy as np

import concourse.bacc as bacc
import concourse.bass as bass
from concourse import mybir
from concourse.bass_utils import run_bass_kernel_spmd

N_CORES = 8
BATCH = 8192
FEAT = 512
NCLS = 2048
P = 128

ROWS = BATCH // N_CORES          # 1024 rows per core
SLOTS = ROWS // P                # 8 rows per partition = 8 gather calls
FREE = SLOTS * FEAT              # 4096 f32 per partition
FHALF = FREE // 2                # feature DMA granularity (2 x 1 MB)

_CACHE: dict[str, object] = {}

F32 = mybir.dt.float32


def _build_nc():
    nc = bacc.Bacc(
        "TRN2", target_bir_lowering=False, debug=False, enable_asserts=False
    )

    feats = nc.dram_tensor("features", [P, FREE], F32, kind="ExternalInput")
    centers = nc.dram_tensor("centers", [NCLS, FEAT], F32, kind="ExternalInput")
    idxs = nc.dram_tensor("idxs", [P, SLOTS], mybir.dt.int32, kind="ExternalInput")
    partials = nc.dram_tensor("partials", [P, SLOTS], F32, kind="ExternalOutput")

    with (
        nc.sbuf_tensor("f_t", [P, FREE], F32) as f_t,
        nc.sbuf_tensor("c_t", [P, FREE], F32) as c_t,
        nc.sbuf_tensor("d_t", [P, FREE], F32) as d_t,
        nc.sbuf_tensor("idx_t", [P, SLOTS], mybir.dt.int32) as idx_t,
        nc.sbuf_tensor("acc", [P, SLOTS], F32) as acc,
        nc.semaphore("s_idx") as s_idx,
        nc.semaphore("s_f0") as s_f0,
        nc.semaphore("s_f1") as s_f1,
        nc.semaphore("s_sub") as s_sub,
        nc.semaphore("s_sq") as s_sq,
        nc.semaphore("s_out") as s_out,
        ExitStack() as stack,
    ):
        # one semaphore per gather DMA: a shared counting sem is racy —
        # per-SDMA-engine completion skew means a cumulative count can hit
        # 16*(g+1) while some engine still owes call g's last bytes
        s_gath = [
            stack.enter_context(nc.semaphore(f"s_g{g}")) for g in range(SLOTS)  # noqa: ANT232
        ]
        s_feat = [s_f0, s_f1]
        block = stack.enter_context(nc.Block())

        @block.sync
        def _(sync: bass.BassEngine):
            # idx first ON THE SAME RING as the features: each SDMA engine
            # drains a ring in FIFO order, so idx's sem increments land ahead
            # of the 2 MB of feature packets (a separate queue would get
            # starved by the round-robin instead)
            sync.dma_start(idx_t[:], idxs[:]).then_inc(s_idx, 16)
            for h in range(2):
                sync.dma_start(
                    f_t[:, h * FHALF:(h + 1) * FHALF],
                    feats[:, h * FHALF:(h + 1) * FHALF],
                ).then_inc(s_feat[h], 16)
            sync.wait_ge(s_sq, SLOTS)
            sync.dma_start(partials[:], acc[:]).then_inc(s_out, 16)
            sync.wait_ge(s_out, 16)

        @block.gpsimd
        def _(gpsimd: bass.BassGpSimd):
            gpsimd.wait_ge(s_idx, 16)
            for g in range(SLOTS):
                gpsimd.indirect_dma_start(
                    out=c_t[:, g * FEAT:(g + 1) * FEAT],
                    out_offset=None,
                    in_=centers[:],
                    in_offset=bass.IndirectOffsetOnAxis(
                        ap=idx_t[:, g:g + 1], axis=0
                    ),
                ).then_inc(s_gath[g], 16)

        @block.vector
        def _(vector: bass.BassEngine):
            for g in range(SLOTS):
                vector.wait_ge(s_gath[g], 16)
                vector.wait_ge(s_feat[g // (SLOTS // 2)], 16)
                vector.tensor_tensor(
                    out=d_t[:, g * FEAT:(g + 1) * FEAT],
                    in0=f_t[:, g * FEAT:(g + 1) * FEAT],
                    in1=c_t[:, g * FEAT:(g + 1) * FEAT],
                    op=mybir.AluOpType.subtract,
                ).then_inc(s_sub, 1)

        @block.scalar
        def _(scalar: bass.BassEngine):
            for g in range(SLOTS):
                scalar.wait_ge(s_sub, g + 1)
                # in-place square: ACT streams read-before-write per element
                scalar.activation(
                    out=d_t[:, g * FEAT:(g + 1) * FEAT],
                    in_=d_t[:, g * FEAT:(g + 1) * FEAT],
                    func=mybir.ActivationFunctionType.Square,
                    accum_out=acc[:, g:g + 1],
                ).then_inc(s_sq, 1)

    nc.compile()
    return nc


def _get_nc():
    if "nc" not in _CACHE:
        _CACHE["nc"] = _build_nc()
    return _CACHE["nc"]


def _prep_inputs(features: np.ndarray, centers: np.ndarray, target: np.ndarray):
    """Shard host-side. Core i takes rows [1024*i, 1024*(i+1)). Within a
    core, rows are ordered by target class and rank k goes to partition
    k % 128, slot k // 128 — each gather call then reads 128 consecutive
    sorted indices, a narrow mostly-sequential window of the centers table
    (much friendlier HBM access than random 2 KB reads)."""
    feats_f32 = np.ascontiguousarray(features, dtype=np.float32).reshape(
        N_CORES, ROWS, FEAT
    )
    tgt = target.astype(np.int32).reshape(N_CORES, ROWS)
    cent = np.ascontiguousarray(centers, dtype=np.float32)

    feats = np.empty((N_CORES, P, FREE), dtype=np.float32)
    idx = np.empty((N_CORES, P, SLOTS), dtype=np.int32)
    for i in range(N_CORES):
        order = np.argsort(tgt[i], kind="stable")
        # rank k -> partition k % P, slot k // P
        feats[i] = (
            feats_f32[i][order].reshape(SLOTS, P, FEAT).transpose(1, 0, 2).reshape(P, FREE)
        )
        idx[i] = tgt[i][order].reshape(SLOTS, P).T
    return feats, cent, idx


def kernel(features: np.ndarray, centers: np.ndarray, target: np.ndarray) -> np.ndarray:
    nc = _get_nc()
    feats, cent, idx = _prep_inputs(features, centers, target)

    in_maps = [
        {"features": feats[i], "centers": cent, "idxs": idx[i]}
        for i in range(N_CORES)
    ]
    res = run_bass_kernel_spmd(nc, in_maps, core_ids=list(range(N_CORES)))

    total = 0.0
    for r in res.results:
        total += float(r["partials"].astype(np.float64).sum())
    loss = total / BATCH + (NCLS - 1) * 1e-12
    return np.asarray(loss, dtype=np.float32)


# revision 17
# speedup vs baseline: 1.1342x; 1.0241x over previous
"""CenterLoss Trainium2 kernel (raw bacc, explicit semaphores).

loss = mean_i clip(||features_i - centers[target_i]||^2, 1e-12, 1e12)
       + (NUM_CLASSES-1) * 1e-12        # the clipped zeros of the masked distmat

The reference builds the full [8192, 2048] distance matrix and masks out
everything but the target column; only the per-row target distance matters,
so the kernel is a gather + (f-c)^2-reduce:

  - data-parallel over the batch: 1024 rows per core on 8 cores
  - centers stay in HBM; per slot g (128 rows, one per partition) an
    indirect SWDGE DMA gathers centers[idx[p, g]] -> c_t[p, g*512:...]
  - DVE computes diff = f - c per slot; ACT squares with fused
    per-partition accumulate into acc[:, g]
  - the per-core [128, 8] partial tiles are summed on the host (the
    "all-reduce" of the scalar loss)

Layout per core: shard row r (0..1023) lives at partition r // 8, slot
r % 8 (the natural contiguous [1024, 512] -> [128, 8*512] reshape);
idx[p, g] = target[8p + g].

Ordering notes (from profiling):
  - the idx load goes first and the feature loads wait for its semaphore —
    otherwise the tiny idx transfer's 16 sem increments trickle out behind
    2 MB of feature packets in the SDMA round-robin and gate the gathers
    ~10 us late
  - indirect_dma_start (InstDMACopy + dynamic AP) gathers one row per
    partition per call; per-call cost is ~1.1 us of Q7 descgen, no
    extended-instruction library load (dma_gather would stall ~6 us on
    LOAD_LIB ucode fetch)
  - ACT's Square is bit-exact for f32 (measured: elementwise == f32
    multiply, accum == f32 sequential sum)
"""

from contextlib import ExitStack

import numpy as np

import concourse.bacc as bacc
import concourse.bass as bass
from concourse import mybir
from concourse.bass_utils import run_bass_kernel_spmd

N_CORES = 8
BATCH = 8192
FEAT = 512
NCLS = 2048
P = 128

ROWS = BATCH // N_CORES          # 1024 rows per core
SLOTS = ROWS // P                # 8 rows per partition = 8 gather calls
FREE = SLOTS * FEAT              # 4096 f32 per partition
FHALF = FREE // 2                # feature DMA granularity (2 x 1 MB)

_CACHE: dict[str, object] = {}

F32 = mybir.dt.float32


def _build_nc():
    nc = bacc.Bacc(
        "TRN2", target_bir_lowering=False, debug=False, enable_asserts=False
    )

    feats = nc.dram_tensor("features", [P, FREE], F32, kind="ExternalInput")
    centers = nc.dram_tensor("centers", [NCLS, FEAT], F32, kind="ExternalInput")
    idxs = nc.dram_tensor("idxs", [P, SLOTS], mybir.dt.int32, kind="ExternalInput")
    partials = nc.dram_tensor("partials", [P, SLOTS], F32, kind="ExternalOutput")

    with (
        nc.sbuf_tensor("f_t", [P, FREE], F32) as f_t,
        nc.sbuf_tensor("c_t", [P, FREE], F32) as c_t,
        nc.sbuf_tensor("d_t", [P, FREE], F32) as d_t,
        nc.sbuf_tensor("idx_t", [P, SLOTS], mybir.dt.int32) as idx_t,
        nc.sbuf_tensor("acc", [P, SLOTS], F32) as acc,
        nc.semaphore("s_idx") as s_idx,
        nc.semaphore("s_f0") as s_f0,
        nc.semaphore("s_f1") as s_f1,
        nc.semaphore("s_sub") as s_sub,
        nc.semaphore("s_sq") as s_sq,
        nc.semaphore("s_out") as s_out,
        ExitStack() as stack,
    ):
        # one semaphore per gather DMA: a shared counting sem is racy —
        # per-SDMA-engine completion skew means a cumulative count can hit
        # 16*(g+1) while some engine still owes call g's last bytes
        s_gath = [
            stack.enter_context(nc.semaphore(f"s_g{g}")) for g in range(SLOTS)  # noqa: ANT232
        ]
        s_feat = [s_f0, s_f1]
        block = stack.enter_context(nc.Block())

        @block.sync
        def _(sync: bass.BassEngine):
            # idx first ON THE SAME RING as the features: each SDMA engine
            # drains a ring in FIFO order, so idx's sem increments land ahead
            # of the 2 MB of feature packets (a separate queue would get
            # starved by the round-robin instead)
            sync.dma_start(idx_t[:], idxs[:]).then_inc(s_idx, 16)
            for h in range(2):
                sync.dma_start(
                    f_t[:, h * FHALF:(h + 1) * FHALF],
                    feats[:, h * FHALF:(h + 1) * FHALF],
                ).then_inc(s_feat[h], 16)
            sync.wait_ge(s_sq, SLOTS)
            sync.dma_start(partials[:], acc[:]).then_inc(s_out, 16)
            sync.wait_ge(s_out, 16)

        @block.gpsimd
        def _(gpsimd: bass.BassGpSimd):
            gpsimd.wait_ge(s_idx, 16)
            for g in range(SLOTS):
                gpsimd.indirect_dma_start(
                    out=c_t[:, g * FEAT:(g + 1) * FEAT],
                    out_offset=None,
                    in_=centers[:],
                    in_offset=bass.IndirectOffsetOnAxis(
                        ap=idx_t[:, g:g + 1], axis=0
                    ),
                ).then_inc(s_gath[g], 16)

        @block.vector
        def _(vector: bass.BassEngine):
            for g in range(SLOTS):
                vector.wait_ge(s_gath[g], 16)
                vector.wait_ge(s_feat[g // (SLOTS // 2)], 16)
                vector.tensor_tensor(
                    out=d_t[:, g * FEAT:(g + 1) * FEAT],
                    in0=f_t[:, g * FEAT:(g + 1) * FEAT],
                    in1=c_t[:, g * FEAT:(g + 1) * FEAT],
                    op=mybir.AluOpType.subtract,
                ).then_inc(s_sub, 1)
            # last slot's square+accum stays on DVE: one fused op right after
            # the last subtract, trimming the ACT handoff + accumulator-read
            # off the critical tail
            g = SLOTS - 1
            vector.scalar_tensor_tensor(
                out=d_t[:, g * FEAT:(g + 1) * FEAT],
                in0=d_t[:, g * FEAT:(g + 1) * FEAT],
                scalar=1.0,
                in1=d_t[:, g * FEAT:(g + 1) * FEAT],
                op0=mybir.AluOpType.mult,
                op1=mybir.AluOpType.mult,
                accum_out=acc[:, g:g + 1],
            ).then_inc(s_sq, 1)

        @block.scalar
        def _(scalar: bass.BassEngine):
            for g in range(SLOTS - 1):
                scalar.wait_ge(s_sub, g + 1)
                # in-place square: ACT streams read-before-write per element
                scalar.activation(
                    out=d_t[:, g * FEAT:(g + 1) * FEAT],
                    in_=d_t[:, g * FEAT:(g + 1) * FEAT],
                    func=mybir.ActivationFunctionType.Square,
                    accum_out=acc[:, g:g + 1],
                ).then_inc(s_sq, 1)

    nc.compile()
    return nc


def _get_nc():
    if "nc" not in _CACHE:
        _CACHE["nc"] = _build_nc()
    return _CACHE["nc"]


def _prep_inputs(features: np.ndarray, centers: np.ndarray, target: np.ndarray):
    """Shard host-side. Core i takes rows [1024*i, 1024*(i+1)). Within a
    core, rows are ordered by target class and rank k goes to partition
    k % 128, slot k // 128 — each gather call then reads 128 consecutive
    sorted indices, a narrow mostly-sequential window of the centers table
    (much friendlier HBM access than random 2 KB reads)."""
    feats_f32 = np.ascontiguousarray(features, dtype=np.float32).reshape(
        N_CORES, ROWS, FEAT
    )
    tgt = target.astype(np.int32).reshape(N_CORES, ROWS)
    cent = np.ascontiguousarray(centers, dtype=np.float32)

    feats = np.empty((N_CORES, P, FREE), dtype=np.float32)
    idx = np.empty((N_CORES, P, SLOTS), dtype=np.int32)
    for i in range(N_CORES):
        order = np.argsort(tgt[i], kind="stable")
        # rank k -> partition k % P, slot k // P
        feats[i] = (
            feats_f32[i][order].reshape(SLOTS, P, FEAT).transpose(1, 0, 2).reshape(P, FREE)
        )
        idx[i] = tgt[i][order].reshape(SLOTS, P).T
    return feats, cent, idx


def kernel(features: np.ndarray, centers: np.ndarray, target: np.ndarray) -> np.ndarray:
    nc = _get_nc()
    feats, cent, idx = _prep_inputs(features, centers, target)

    in_maps = [
        {"features": feats[i], "centers": cent, "idxs": idx[i]}
        for i in range(N_CORES)
    ]
    res = run_bass_kernel_spmd(nc, in_maps, core_ids=list(range(N_CORES)))

    total = 0.0
    for r in res.results:
        total += float(r["partials"].astype(np.float64).sum())
    loss = total / BATCH + (NCLS - 1) * 1e-12
    return np.asarray(loss, dtype=np.float32)
